# revision 30
# baseline (speedup 1.0000x reference)
"""Trainium2 Bass kernel for MultiHeadAttention with relative position bias.

Reference computation (B=2, S=2048, D=1024, H=16, Dk=64, MAX_REL=128):
    Q,K,V = x@W{q,k,v}.T + b      (per-head reshape)
    scores = QK^T/sqrt(Dk) + rel_bias_matrix
    out = softmax(scores) @ V, heads merged, @ Wo.T + bo

Sharding (8 cores): core c handles batch b=c//4 and 4 heads hg=4*(c%4)..+4
(data + head parallel). Q/K/V projections column-split per head group,
Wo row-split; the partial outputs are summed on the host (the "all-reduce").

v2 design (ACT-bound pipeline, ~2x over v1):
  The exp stream on the Scalar/ACT engine (16.8M elem/core at 1 elem/
  cycle/lane @1.2GHz) is the hard floor (~135us); everything else hides
  under it.  All tensor data is bf16 (final rel err ~6.5e-3 vs 2e-2
  tolerance): halves input DMA (startup), enables FWL weight loads, and
  shrinks SBUF.  PSUM budget: stp pool (2x[128,1024] slots, 4 banks) for
  QK scores + Wo tiles; accp pool (2 slots, 4 banks) for projection
  flights + PV accumulators.

  Emission order: interleaved per-dc DMAs (wk,wq,xt) -> K0/Q0 flights ->
  group0 QK+exp (ACT starts ~15us, pt tiles buffered deep) -> K1/Q1/V
  flights -> group0 PV + normalize -> groups 1-3 fully interleaved ->
  Wo(qh0) -> Wo(qh1).  Group order is qh-major: (p0,q0),(p1,q0),(p0,q1),
  (p1,q1).

  Softmax trick (from v1): P~ = exp(s/8) is the softmax numerator up to
  the per-head constant e^{-c_past} which cancels in the normalize; the
  "future" region (q-k <= -128) gets a constant multiplier on DVE and
  the 384-wide Toeplitz band a host-precomputed exp(bias - c_past) tile.
  V carries a ones column so PV yields the denominator for free;
  normalize uses reciprocal_approx_fast (5x DVE reciprocal) on a
  DRAM-broadcast denominator.
"""

import math
import os
import sys

for _p in ("/opt/trn_rl_repo", "/root/.axon_site", "/root/.axon_site/_ro/trn_rl_repo",
           "/root/.axon_site/_ro/pypackages"):
    if os.path.isdir(_p) and _p not in sys.path:
        sys.path.append(_p)

import numpy as np
import ml_dtypes

import concourse.bass as bass
import concourse.mybir as mybir
import concourse.tile as tile
from concourse import bacc
from contextlib import ExitStack

# Problem constants (hardcoded per the contract).
B, S, D = 2, 2048, 1024
H, DK = 16, 64
MAX_REL = 128
N_CORES = 8
CORES_PER_BATCH = 4
HEADS_PER_CORE = H // CORES_PER_BATCH  # 4
CL = HEADS_PER_CORE * DK               # 256 local channels
N_PAIRS = HEADS_PER_CORE // 2          # 2 head pairs
QH = 1024                              # q processed in halves
N_QH = S // QH                         # 2
N_KC = S // 128                        # 16 k chunks
NDC = D // 128                         # 8 contraction chunks
BAND = 3 * 128                         # band width in q for one k chunk

F32 = mybir.dt.float32
BF16 = mybir.dt.bfloat16

SCALE = 1.0 / math.sqrt(DK)
EXP = mybir.ActivationFunctionType.Exp


def build_program():
    nc = bacc.Bacc("TRN2", target_bir_lowering=False, debug=False)

    xt_d = nc.declare_dram_parameter("xt", [D, S], BF16, isOutput=False)
    # q/k/v weights packed in one tensor: one DMA config (~700ns Sync each)
    # and one contiguous ~1.5MB transfer covers all three projections
    wqkv_d = nc.declare_dram_parameter("wqkv", [D, 3, CL], BF16, isOutput=False)
    wot_d = nc.declare_dram_parameter("wot", [CL, D], BF16, isOutput=False)
    # partial output, TRANSPOSED [D, S] in bf16 (host upcasts + transposes)
    bqk_d = nc.declare_dram_parameter("bqk", [128, 4], F32, isOutput=False)
    band_d = nc.declare_dram_parameter("band", [128, HEADS_PER_CORE, BAND], BF16,
                                       isOutput=False)
    # future-region multiplier exp(c_fut - c_past), replicated over partitions
    fmult_d = nc.declare_dram_parameter("fmult", [128, HEADS_PER_CORE], F32,
                                        isOutput=False)
    out_d = nc.declare_dram_parameter("out_p", [D, S], BF16, isOutput=True)
    # denominator round-trip scratch: [group, hh, q]
    den_d = nc.dram_tensor("den_scratch", [2 * N_PAIRS * N_QH, 2, QH], F32)

    with tile.TileContext(nc) as tc, ExitStack() as ctx:
        # ---------- long-lived SBUF ----------
        persist = ctx.enter_context(tc.tile_pool(name="persist", bufs=1))
        q_sb = persist.tile([128, 2, S], BF16, tag="q_sb")
        k_sb = persist.tile([128, 2, S], BF16, tag="k_sb")
        v_sb = persist.tile([128, N_KC, HEADS_PER_CORE, DK + 1], BF16, tag="v_sb")
        ct_sb = persist.tile([128, 2, S], BF16, tag="ct_sb")
        wo_sb = persist.tile([128, 2, D], BF16, tag="wo_sb")
        band_sb = persist.tile([128, HEADS_PER_CORE, BAND], BF16, tag="band_sb")
        bqk_sb = persist.tile([128, 4], F32, tag="bqk_sb")
        fmult_sb = persist.tile([128, HEADS_PER_CORE], F32, tag="fmult_sb")
        warm_sb = persist.tile([128, 16], F32, tag="warm_sb")

        xw = ctx.enter_context(tc.tile_pool(name="xw", bufs=1))
        xt_sb = xw.tile([128, NDC, S], BF16, tag="xt_sb")
        wqkv_sb = xw.tile([128, NDC, 3, CL], BF16, tag="wqkv_sb")

        nrm = ctx.enter_context(tc.tile_pool(name="nrm", bufs=2))
        ptp = ctx.enter_context(tc.tile_pool(name="ptp", bufs=17))
        outp = ctx.enter_context(tc.tile_pool(name="outp", bufs=3))

        # ---------- PSUM pools: 4 banks each ----------
        stp = ctx.enter_context(tc.tile_pool(name="stp", bufs=2, space="PSUM"))
        accp = ctx.enter_context(tc.tile_pool(name="accp", bufs=2, space="PSUM"))

        # ---------- input DMAs, ordered by consumption ----------
        xt_v = xt_d.ap().rearrange("(c p) s -> p c s", p=128)
        wqkv_v = wqkv_d.ap().rearrange("(c p) g m -> p c g m", p=128)

        # ACT exp-table warmup (overlaps the input DMA wait)
        nc.vector.memset(warm_sb, 0.0)
        nc.scalar.activation(out=warm_sb, in_=warm_sb, func=EXP, scale=1.0)

        # Few, large DMAs (each dma_start costs ~700ns of Sync config and the
        # transfers fan out over all queues anyway): weights, then the s<1024
        # half of xt (unblocks K0/Q0 t=0 and attention group 0), the small
        # tables, then the rest.
        nc.sync.dma_start(out=wqkv_sb, in_=wqkv_v)
        nc.sync.dma_start(out=xt_sb[:, :, 0:1024],
                          in_=xt_v[:, :, 0:1024])
        nc.sync.dma_start(out=band_sb, in_=band_d.ap())
        nc.sync.dma_start(out=bqk_sb, in_=bqk_d.ap())
        nc.sync.dma_start(out=fmult_sb, in_=fmult_d.ap())
        nc.sync.dma_start(out=xt_sb[:, :, 1024:2048],
                          in_=xt_v[:, :, 1024:2048])
        nc.sync.dma_start(out=wo_sb, in_=wot_d.ap().rearrange("(c p) m -> p c m", p=128))
        nc.vector.memset(v_sb[:, :, :, DK:DK + 1], 1.0)

        # ---------- projection flights (dc-outer: LDW amortized 4x) ----------
        def qk_flight_t(g, dst_sb, j, bias_col, t):
            """One s-half of a Q/K projection: finer emission granularity so
            attention group 0 can start before the rest of the projections."""
            slot = accp.tile([128, 1024], F32, tag="acc")
            for dc in range(NDC):
                for half in range(2):
                    nc.tensor.matmul(
                        slot[:, half * 512:(half + 1) * 512],
                        lhsT=wqkv_sb[:, dc, g, j * 128:(j + 1) * 128],
                        rhs=xt_sb[:, dc, t * 1024 + half * 512:
                                  t * 1024 + (half + 1) * 512],
                        start=(dc == 0), stop=(dc == NDC - 1),
                    )
            nc.vector.tensor_scalar_add(
                out=dst_sb[:, j, t * 1024:(t + 1) * 1024],
                in0=slot,
                scalar1=bqk_sb[:, bias_col:bias_col + 1],
            )

        def v_flight(scg):
            # each 256-wide accumulation group must own a full PSUM bank
            # (start=True clears has_written for the whole bank), so 4
            # s-chunks land at 512-col boundaries across two slots.
            slot_a = accp.tile([128, 1024], F32, tag="acc")
            slot_b = accp.tile([128, 1024], F32, tag="acc")
            both = (slot_a, slot_b)
            for dc in range(NDC):
                for i in range(4):
                    sc = scg * 4 + i
                    nc.tensor.matmul(
                        both[i // 2][:, (i % 2) * 512:(i % 2) * 512 + CL],
                        lhsT=xt_sb[:, dc, sc * 128:(sc + 1) * 128],
                        rhs=wqkv_sb[:, dc, 2, :],
                        start=(dc == 0), stop=(dc == NDC - 1),
                    )
            for i in range(4):
                sc = scg * 4 + i
                nc.scalar.copy(
                    out=v_sb[:, sc, :, 0:DK],
                    in_=both[i // 2][:, (i % 2) * 512:(i % 2) * 512 + CL]
                    .rearrange("p (h d) -> p h d", h=HEADS_PER_CORE),
                )

        # ---------- attention pieces ----------
        def qk_exp(pair, qh, kc):
            """QK matmuls + exp + band/future fixups; returns the pt tile."""
            w0 = qh * QH
            k0 = kc * 128
            pt = ptp.tile([128, 2, QH], BF16, tag="pt")
            for hh in range(2):
                h = 2 * pair + hh
                p0 = hh * 64
                st = stp.tile([128, QH], F32, tag="st")
                for half in range(2):
                    nc.tensor.matmul(
                        st[:, half * 512:(half + 1) * 512],
                        lhsT=k_sb[p0:p0 + 64, pair, k0:k0 + 128],
                        rhs=q_sb[p0:p0 + 64, pair,
                                 w0 + half * 512:w0 + (half + 1) * 512],
                        start=True, stop=True,
                        tile_position=(p0, 0),
                    )
                nc.scalar.activation(out=pt[:, hh, :], in_=st, func=EXP,
                                     scale=SCALE)
                # future region (q <= k0-129): multiply by exp(c_fut - c_past)
                fut_end = min(max(k0 - 128, w0), w0 + QH)
                n_fut = fut_end - w0
                if n_fut > 0:
                    nc.vector.tensor_scalar_mul(
                        out=pt[:, hh, 0:n_fut], in0=pt[:, hh, 0:n_fut],
                        scalar1=fmult_sb[:, h:h + 1],
                    )
                # band: q in [k0-128, k0+256) -> multiply exp(bias - c_past)
                b_lo = max(k0 - 128, w0)
                b_hi = min(k0 + 2 * 128, w0 + QH)
                if b_hi > b_lo:
                    m0 = b_lo - (k0 - 128)
                    nc.vector.tensor_mul(
                        out=pt[:, hh, b_lo - w0:b_hi - w0],
                        in0=pt[:, hh, b_lo - w0:b_hi - w0],
                        in1=band_sb[:, h, m0:m0 + (b_hi - b_lo)],
                    )
            return pt

        def pv(pair, kc, pt, accs):
            for hh in range(2):
                for sub in range(2):
                    nc.tensor.matmul(
                        accs[hh][:, sub * 512:(sub + 1) * 512],
                        lhsT=v_sb[:, kc, 2 * pair + hh, :],
                        rhs=pt[:, hh, sub * 512:(sub + 1) * 512],
                        start=(kc == 0), stop=(kc == N_KC - 1),
                    )

        def evict_and_normalize(gi, pair, qh, accs):
            w0 = qh * QH
            den_sb = nrm.tile([1, 2, QH], F32, tag="den")
            # hh=0 eviction on ACT, hh=1 on DVE: the two run in parallel so
            # the PV accumulator slots release ~1us sooner for the next group
            nc.scalar.copy(
                out=ct_sb[0:64, pair, w0:w0 + QH], in_=accs[0][0:DK, :])
            nc.vector.tensor_copy(
                out=den_sb[:, 0, :], in_=accs[0][DK:DK + 1, :])
            nc.vector.tensor_copy(
                out=ct_sb[64:128, pair, w0:w0 + QH], in_=accs[1][0:DK, :])
            nc.vector.tensor_copy(
                out=den_sb[:, 1, :], in_=accs[1][DK:DK + 1, :])
            den_v = den_d.ap()
            nc.sync.dma_start(out=den_v[gi], in_=den_sb)
            rbc = nrm.tile([128, QH], F32, tag="rbc")
            for hh in range(2):
                bsrc = bass.AP(
                    tensor=den_v.tensor,
                    offset=den_v.offset + (gi * 2 + hh) * QH,
                    ap=[[0, 64], [1, QH]],
                )
                nc.sync.dma_start(out=rbc[hh * 64:hh * 64 + 64, :], in_=bsrc)
            nc.vector.reciprocal_approx_fast(out=rbc, in_=rbc)
            nc.vector.tensor_mul(
                out=ct_sb[:, pair, w0:w0 + QH],
                in0=ct_sb[:, pair, w0:w0 + QH],
                in1=rbc,
            )

        def wo_phase():
            # transposed output out^T[m, s]: Wo slices are the stationary
            # operand (LDW amortized 4x: one load covers both s-halves), ct
            # streams.  Evictions on ACT (idle in the tail), bf16 partials
            # halve the out-DMA bytes.
            for mt in range(8):
                ps0 = stp.tile([128, 1024], F32, tag="st")
                ps1 = stp.tile([128, 1024], F32, tag="st")
                both = (ps0, ps1)
                for j in range(2):
                    for sh in range(2):
                        for ss in range(2):
                            nc.tensor.matmul(
                                both[sh][:, ss * 512:(ss + 1) * 512],
                                lhsT=wo_sb[:, j, mt * 128:(mt + 1) * 128],
                                rhs=ct_sb[:, j, sh * 1024 + ss * 512:
                                          sh * 1024 + (ss + 1) * 512],
                                start=(j == 0), stop=(j == 1),
                            )
                for sh in range(2):
                    o_sb = outp.tile([128, 1024], BF16, tag="o_sb")
                    nc.scalar.copy(out=o_sb, in_=both[sh])
                    nc.sync.dma_start(
                        out=out_d.ap()[mt * 128:(mt + 1) * 128,
                                       sh * 1024:(sh + 1) * 1024],
                        in_=o_sb)

        # ---------- emission schedule ----------
        # group order qh-major: (p0,q0), (p1,q0), (p0,q1), (p1,q1)
        GROUPS = [(0, 0), (1, 0), (0, 1), (1, 1)]

        # K0/Q0 t=0 unblock attention group 0 after only a quarter of the
        # Q/K projection work (and only the s<1024 xt halves).
        pair0, qh0 = GROUPS[0]
        g0_pts = []
        qk_flight_t(1, k_sb, 0, 2, 0)
        qk_flight_t(0, q_sb, 0, 0, 0)
        for kc in range(8):
            g0_pts.append(qk_exp(pair0, qh0, kc))
        qk_flight_t(1, k_sb, 0, 2, 1)
        for kc in range(8, N_KC):
            g0_pts.append(qk_exp(pair0, qh0, kc))
        qk_flight_t(0, q_sb, 0, 0, 1)
        qk_flight_t(1, k_sb, 1, 3, 0)
        qk_flight_t(1, k_sb, 1, 3, 1)
        qk_flight_t(0, q_sb, 1, 1, 0)
        qk_flight_t(0, q_sb, 1, 1, 1)
        for scg in range(4):
            v_flight(scg)

        # group 0 PV + normalize
        acc_a = accp.tile([DK + 1, QH], F32, tag="acc")
        acc_b = accp.tile([DK + 1, QH], F32, tag="acc")
        accs = [acc_a, acc_b]
        for kc in range(N_KC):
            pv(pair0, kc, g0_pts[kc], accs)
        evict_and_normalize(0, pair0, qh0, accs)

        # groups 1-3: fully interleaved steady state
        for gi in range(1, 4):
            pair, qh = GROUPS[gi]
            acc_a = accp.tile([DK + 1, QH], F32, tag="acc")
            acc_b = accp.tile([DK + 1, QH], F32, tag="acc")
            accs = [acc_a, acc_b]
            for kc in range(N_KC):
                pt = qk_exp(pair, qh, kc)
                pv(pair, kc, pt, accs)
            evict_and_normalize(gi, pair, qh, accs)

        wo_phase()

    nc.compile()
    return nc


def make_core_inputs(x, Wq, bq, Wk, bk, Wv, bv, Wo, bo, rel_bias):
    """Host-side shard prep. Returns list of 8 in_maps."""
    bf16 = ml_dtypes.bfloat16
    x = np.asarray(x, np.float32)
    in_maps = []
    WqT = np.ascontiguousarray(np.asarray(Wq, np.float32).T.astype(bf16))
    WkT = np.ascontiguousarray(np.asarray(Wk, np.float32).T.astype(bf16))
    WvT = np.ascontiguousarray(np.asarray(Wv, np.float32).T.astype(bf16))
    WoT = np.ascontiguousarray(np.asarray(Wo, np.float32).T.astype(bf16))
    rel = np.asarray(rel_bias, np.float32)
    xt = [np.ascontiguousarray(x[b].T.astype(bf16)) for b in range(B)]

    # band multiplier: [p, h_local, m] = exp(bias(q,k) - c_past), q-k = m-128-p
    p_i = np.arange(128)[:, None]
    m_i = np.arange(BAND)[None, :]
    delta = np.clip(m_i - 128 - p_i, -MAX_REL, MAX_REL) + MAX_REL  # [128, 384]

    for c in range(N_CORES):
        b = c // CORES_PER_BATCH
        g = c % CORES_PER_BATCH
        c0 = g * CL
        heads = np.arange(g * HEADS_PER_CORE, (g + 1) * HEADS_PER_CORE)

        bqk = np.empty((128, 4), np.float32)
        bqk[:, 0] = np.asarray(bq, np.float32)[c0:c0 + 128]
        bqk[:, 1] = np.asarray(bq, np.float32)[c0 + 128:c0 + 256]
        bqk[:, 2] = np.asarray(bk, np.float32)[c0:c0 + 128]
        bqk[:, 3] = np.asarray(bk, np.float32)[c0 + 128:c0 + 256]

        band = np.empty((128, HEADS_PER_CORE, BAND), np.float32)
        fmult = np.empty((128, HEADS_PER_CORE), np.float32)
        for i, hg in enumerate(heads):
            c_past = rel[hg, 2 * MAX_REL]
            band[:, i, :] = np.exp(rel[hg][delta] - c_past)
            fmult[:, i] = np.exp(rel[hg, 0] - c_past)  # future multiplier
        wqkv = np.stack([WqT[:, c0:c0 + CL], WkT[:, c0:c0 + CL],
                         WvT[:, c0:c0 + CL]], axis=1)
        in_maps.append({
            "xt": xt[b],
            "wqkv": np.ascontiguousarray(wqkv),
            "wot": np.ascontiguousarray(WoT[c0:c0 + CL, :]),
            "bqk": bqk,
            "band": band.astype(bf16),
            "fmult": fmult,
        })
    return in_maps


_NC_CACHE = {}


def get_program(**kw):
    key = tuple(sorted(kw.items()))
    if key not in _NC_CACHE:
        _NC_CACHE[key] = build_program(**kw)
    return _NC_CACHE[key]


def kernel(x, Wq, bq, Wk, bk, Wv, bv, Wo, bo, rel_bias):
    from concourse.bass_utils import run_bass_kernel_spmd

    nc = get_program()
    in_maps = make_core_inputs(x, Wq, bq, Wk, bk, Wv, bv, Wo, bo, rel_bias)
    res = run_bass_kernel_spmd(nc, in_maps, core_ids=list(range(N_CORES)))
    results = res.results

    Wo_np = np.asarray(Wo, np.float32)
    const = np.asarray(bv, np.float32) @ Wo_np.T + np.asarray(bo, np.float32)
    out = np.zeros((B, S, D), np.float32)
    for c in range(N_CORES):
        out[c // CORES_PER_BATCH] += results[c]["out_p"].astype(np.float32).T
    out += const[None, None, :]
    return out


# revision 31
# speedup vs baseline: 1.0171x; 1.0171x over previous
"""Trainium2 Bass kernel for MultiHeadAttention with relative position bias.

Reference computation (B=2, S=2048, D=1024, H=16, Dk=64, MAX_REL=128):
    Q,K,V = x@W{q,k,v}.T + b      (per-head reshape)
    scores = QK^T/sqrt(Dk) + rel_bias_matrix
    out = softmax(scores) @ V, heads merged, @ Wo.T + bo

Sharding (8 cores): core c handles batch b=c//4 and 4 heads hg=4*(c%4)..+4
(data + head parallel). Q/K/V projections column-split per head group,
Wo row-split; the partial outputs are summed on the host (the "all-reduce").

v2 design (ACT-bound pipeline, ~2x over v1):
  The exp stream on the Scalar/ACT engine (16.8M elem/core at 1 elem/
  cycle/lane @1.2GHz) is the hard floor (~135us); everything else hides
  under it.  All tensor data is bf16 (final rel err ~6.5e-3 vs 2e-2
  tolerance): halves input DMA (startup), enables FWL weight loads, and
  shrinks SBUF.  PSUM budget: stp pool (2x[128,1024] slots, 4 banks) for
  QK scores + Wo tiles; accp pool (2 slots, 4 banks) for projection
  flights + PV accumulators.

  Emission order: interleaved per-dc DMAs (wk,wq,xt) -> K0/Q0 flights ->
  group0 QK+exp (ACT starts ~15us, pt tiles buffered deep) -> K1/Q1/V
  flights -> group0 PV + normalize -> groups 1-3 fully interleaved ->
  Wo(qh0) -> Wo(qh1).  Group order is qh-major: (p0,q0),(p1,q0),(p0,q1),
  (p1,q1).

  Softmax trick (from v1): P~ = exp(s/8) is the softmax numerator up to
  the per-head constant e^{-c_past} which cancels in the normalize; the
  "future" region (q-k <= -128) gets a constant multiplier on DVE and
  the 384-wide Toeplitz band a host-precomputed exp(bias - c_past) tile.
  V carries a ones column so PV yields the denominator for free;
  normalize uses reciprocal_approx_fast (5x DVE reciprocal) on a
  DRAM-broadcast denominator.
"""

import math
import os
import sys

for _p in ("/opt/trn_rl_repo", "/root/.axon_site", "/root/.axon_site/_ro/trn_rl_repo",
           "/root/.axon_site/_ro/pypackages"):
    if os.path.isdir(_p) and _p not in sys.path:
        sys.path.append(_p)

import numpy as np
import ml_dtypes

import concourse.bass as bass
import concourse.mybir as mybir
import concourse.tile as tile
from concourse import bacc
from contextlib import ExitStack

# Problem constants (hardcoded per the contract).
B, S, D = 2, 2048, 1024
H, DK = 16, 64
MAX_REL = 128
N_CORES = 8
CORES_PER_BATCH = 4
HEADS_PER_CORE = H // CORES_PER_BATCH  # 4
CL = HEADS_PER_CORE * DK               # 256 local channels
N_PAIRS = HEADS_PER_CORE // 2          # 2 head pairs
QH = 1024                              # q processed in halves
N_QH = S // QH                         # 2
N_KC = S // 128                        # 16 k chunks
NDC = D // 128                         # 8 contraction chunks
BAND = 3 * 128                         # band width in q for one k chunk

F32 = mybir.dt.float32
BF16 = mybir.dt.bfloat16

SCALE = 1.0 / math.sqrt(DK)
EXP = mybir.ActivationFunctionType.Exp


def build_program():
    nc = bacc.Bacc("TRN2", target_bir_lowering=False, debug=False)

    xt_d = nc.declare_dram_parameter("xt", [D, S], BF16, isOutput=False)
    # q/k/v weights packed in one tensor: one DMA config (~700ns Sync each)
    # and one contiguous ~1.5MB transfer covers all three projections
    wqkv_d = nc.declare_dram_parameter("wqkv", [D, 3, CL], BF16, isOutput=False)
    wot_d = nc.declare_dram_parameter("wot", [CL, D], BF16, isOutput=False)
    # partial output, TRANSPOSED [D, S] in bf16 (host upcasts + transposes)
    bqk_d = nc.declare_dram_parameter("bqk", [128, 4], F32, isOutput=False)
    band_d = nc.declare_dram_parameter("band", [128, HEADS_PER_CORE, BAND], BF16,
                                       isOutput=False)
    # future-region multiplier exp(c_fut - c_past), replicated over partitions
    fmult_d = nc.declare_dram_parameter("fmult", [128, HEADS_PER_CORE], F32,
                                        isOutput=False)
    out_d = nc.declare_dram_parameter("out_p", [D, S], BF16, isOutput=True)
    # denominator round-trip scratch: [group, hh, q]
    den_d = nc.dram_tensor("den_scratch", [2 * N_PAIRS * N_QH, 2, QH], F32)

    with tile.TileContext(nc) as tc, ExitStack() as ctx:
        # ---------- long-lived SBUF ----------
        persist = ctx.enter_context(tc.tile_pool(name="persist", bufs=1))
        q_sb = persist.tile([128, 2, S], BF16, tag="q_sb")
        k_sb = persist.tile([128, 2, S], BF16, tag="k_sb")
        v_sb = persist.tile([128, N_KC, HEADS_PER_CORE, DK + 1], BF16, tag="v_sb")
        ct_sb = persist.tile([128, 2, S], BF16, tag="ct_sb")
        wo_sb = persist.tile([128, 2, D], BF16, tag="wo_sb")
        band_sb = persist.tile([128, HEADS_PER_CORE, BAND], BF16, tag="band_sb")
        bqk_sb = persist.tile([128, 4], F32, tag="bqk_sb")
        fmult_sb = persist.tile([128, HEADS_PER_CORE], F32, tag="fmult_sb")
        warm_sb = persist.tile([128, 16], F32, tag="warm_sb")

        xw = ctx.enter_context(tc.tile_pool(name="xw", bufs=1))
        xt_sb = xw.tile([128, NDC, S], BF16, tag="xt_sb")
        wqkv_sb = xw.tile([128, NDC, 3, CL], BF16, tag="wqkv_sb")

        nrm = ctx.enter_context(tc.tile_pool(name="nrm", bufs=2))
        ptp = ctx.enter_context(tc.tile_pool(name="ptp", bufs=17))
        outp = ctx.enter_context(tc.tile_pool(name="outp", bufs=3))

        # ---------- PSUM pools: 4 banks each ----------
        stp = ctx.enter_context(tc.tile_pool(name="stp", bufs=2, space="PSUM"))
        accp = ctx.enter_context(tc.tile_pool(name="accp", bufs=2, space="PSUM"))

        # ---------- input DMAs, ordered by consumption ----------
        xt_v = xt_d.ap().rearrange("(c p) s -> p c s", p=128)
        wqkv_v = wqkv_d.ap().rearrange("(c p) g m -> p c g m", p=128)

        # ACT exp-table warmup (overlaps the input DMA wait)
        nc.vector.memset(warm_sb, 0.0)
        nc.scalar.activation(out=warm_sb, in_=warm_sb, func=EXP, scale=1.0)

        # Medium-grain DMAs: one dma_start lands on ~one queue (~24 GB/s), so
        # parallelism comes from multiple in-flight dma_starts; order matches
        # consumption (wk/wq halves -> xt s<1024 chunks -> tables -> rest).
        for g, half in ((1, 0), (1, 1), (0, 0), (0, 1)):   # wk then wq
            nc.sync.dma_start(out=wqkv_sb[:, half * 4:(half + 1) * 4, g, :],
                              in_=wqkv_v[:, half * 4:(half + 1) * 4, g, :])
        for dc in range(NDC):
            nc.sync.dma_start(out=xt_sb[:, dc, 0:1024],
                              in_=xt_v[:, dc, 0:1024])
        nc.sync.dma_start(out=band_sb, in_=band_d.ap())
        nc.sync.dma_start(out=bqk_sb, in_=bqk_d.ap())
        nc.sync.dma_start(out=fmult_sb, in_=fmult_d.ap())
        for dc in range(NDC):
            nc.sync.dma_start(out=xt_sb[:, dc, 1024:2048],
                              in_=xt_v[:, dc, 1024:2048])
        for half in range(2):                               # wv
            nc.sync.dma_start(out=wqkv_sb[:, half * 4:(half + 1) * 4, 2, :],
                              in_=wqkv_v[:, half * 4:(half + 1) * 4, 2, :])
        nc.sync.dma_start(out=wo_sb, in_=wot_d.ap().rearrange("(c p) m -> p c m", p=128))
        nc.vector.memset(v_sb[:, :, :, DK:DK + 1], 1.0)

        # ---------- projection flights (dc-outer: LDW amortized 4x) ----------
        def qk_flight_t(g, dst_sb, j, bias_col, t):
            """One s-half of a Q/K projection: finer emission granularity so
            attention group 0 can start before the rest of the projections."""
            slot = accp.tile([128, 1024], F32, tag="acc")
            for dc in range(NDC):
                for half in range(2):
                    nc.tensor.matmul(
                        slot[:, half * 512:(half + 1) * 512],
                        lhsT=wqkv_sb[:, dc, g, j * 128:(j + 1) * 128],
                        rhs=xt_sb[:, dc, t * 1024 + half * 512:
                                  t * 1024 + (half + 1) * 512],
                        start=(dc == 0), stop=(dc == NDC - 1),
                    )
            nc.vector.tensor_scalar_add(
                out=dst_sb[:, j, t * 1024:(t + 1) * 1024],
                in0=slot,
                scalar1=bqk_sb[:, bias_col:bias_col + 1],
            )

        def v_flight(scg):
            # each 256-wide accumulation group must own a full PSUM bank
            # (start=True clears has_written for the whole bank), so 4
            # s-chunks land at 512-col boundaries across two slots.
            slot_a = accp.tile([128, 1024], F32, tag="acc")
            slot_b = accp.tile([128, 1024], F32, tag="acc")
            both = (slot_a, slot_b)
            for dc in range(NDC):
                for i in range(4):
                    sc = scg * 4 + i
                    nc.tensor.matmul(
                        both[i // 2][:, (i % 2) * 512:(i % 2) * 512 + CL],
                        lhsT=xt_sb[:, dc, sc * 128:(sc + 1) * 128],
                        rhs=wqkv_sb[:, dc, 2, :],
                        start=(dc == 0), stop=(dc == NDC - 1),
                    )
            for i in range(4):
                sc = scg * 4 + i
                nc.scalar.copy(
                    out=v_sb[:, sc, :, 0:DK],
                    in_=both[i // 2][:, (i % 2) * 512:(i % 2) * 512 + CL]
                    .rearrange("p (h d) -> p h d", h=HEADS_PER_CORE),
                )

        # ---------- attention pieces ----------
        def qk_exp(pair, qh, kc):
            """QK matmuls + exp + band/future fixups; returns the pt tile."""
            w0 = qh * QH
            k0 = kc * 128
            pt = ptp.tile([128, 2, QH], BF16, tag="pt")
            for hh in range(2):
                h = 2 * pair + hh
                p0 = hh * 64
                st = stp.tile([128, QH], F32, tag="st")
                for half in range(2):
                    nc.tensor.matmul(
                        st[:, half * 512:(half + 1) * 512],
                        lhsT=k_sb[p0:p0 + 64, pair, k0:k0 + 128],
                        rhs=q_sb[p0:p0 + 64, pair,
                                 w0 + half * 512:w0 + (half + 1) * 512],
                        start=True, stop=True,
                        tile_position=(p0, 0),
                    )
                nc.scalar.activation(out=pt[:, hh, :], in_=st, func=EXP,
                                     scale=SCALE)
                # future region (q <= k0-129): multiply by exp(c_fut - c_past)
                fut_end = min(max(k0 - 128, w0), w0 + QH)
                n_fut = fut_end - w0
                if n_fut > 0:
                    nc.vector.tensor_scalar_mul(
                        out=pt[:, hh, 0:n_fut], in0=pt[:, hh, 0:n_fut],
                        scalar1=fmult_sb[:, h:h + 1],
                    )
                # band: q in [k0-128, k0+256) -> multiply exp(bias - c_past)
                b_lo = max(k0 - 128, w0)
                b_hi = min(k0 + 2 * 128, w0 + QH)
                if b_hi > b_lo:
                    m0 = b_lo - (k0 - 128)
                    nc.vector.tensor_mul(
                        out=pt[:, hh, b_lo - w0:b_hi - w0],
                        in0=pt[:, hh, b_lo - w0:b_hi - w0],
                        in1=band_sb[:, h, m0:m0 + (b_hi - b_lo)],
                    )
            return pt

        def pv(pair, kc, pt, accs):
            for hh in range(2):
                for sub in range(2):
                    nc.tensor.matmul(
                        accs[hh][:, sub * 512:(sub + 1) * 512],
                        lhsT=v_sb[:, kc, 2 * pair + hh, :],
                        rhs=pt[:, hh, sub * 512:(sub + 1) * 512],
                        start=(kc == 0), stop=(kc == N_KC - 1),
                    )

        def evict_and_normalize(gi, pair, qh, accs):
            w0 = qh * QH
            den_sb = nrm.tile([1, 2, QH], F32, tag="den")
            # hh=0 eviction on ACT, hh=1 on DVE: the two run in parallel so
            # the PV accumulator slots release ~1us sooner for the next group
            nc.scalar.copy(
                out=ct_sb[0:64, pair, w0:w0 + QH], in_=accs[0][0:DK, :])
            nc.vector.tensor_copy(
                out=den_sb[:, 0, :], in_=accs[0][DK:DK + 1, :])
            nc.vector.tensor_copy(
                out=ct_sb[64:128, pair, w0:w0 + QH], in_=accs[1][0:DK, :])
            nc.vector.tensor_copy(
                out=den_sb[:, 1, :], in_=accs[1][DK:DK + 1, :])
            den_v = den_d.ap()
            nc.sync.dma_start(out=den_v[gi], in_=den_sb)
            rbc = nrm.tile([128, QH], F32, tag="rbc")
            for hh in range(2):
                bsrc = bass.AP(
                    tensor=den_v.tensor,
                    offset=den_v.offset + (gi * 2 + hh) * QH,
                    ap=[[0, 64], [1, QH]],
                )
                nc.sync.dma_start(out=rbc[hh * 64:hh * 64 + 64, :], in_=bsrc)
            nc.vector.reciprocal_approx_fast(out=rbc, in_=rbc)
            nc.vector.tensor_mul(
                out=ct_sb[:, pair, w0:w0 + QH],
                in0=ct_sb[:, pair, w0:w0 + QH],
                in1=rbc,
            )

        def wo_phase():
            # transposed output out^T[m, s]: Wo slices are the stationary
            # operand (LDW amortized 4x: one load covers both s-halves), ct
            # streams.  Evictions on ACT (idle in the tail), bf16 partials
            # halve the out-DMA bytes.
            for mt in range(8):
                ps0 = stp.tile([128, 1024], F32, tag="st")
                ps1 = stp.tile([128, 1024], F32, tag="st")
                both = (ps0, ps1)
                for j in range(2):
                    for sh in range(2):
                        for ss in range(2):
                            nc.tensor.matmul(
                                both[sh][:, ss * 512:(ss + 1) * 512],
                                lhsT=wo_sb[:, j, mt * 128:(mt + 1) * 128],
                                rhs=ct_sb[:, j, sh * 1024 + ss * 512:
                                          sh * 1024 + (ss + 1) * 512],
                                start=(j == 0), stop=(j == 1),
                            )
                for sh in range(2):
                    o_sb = outp.tile([128, 1024], BF16, tag="o_sb")
                    nc.scalar.copy(out=o_sb, in_=both[sh])
                    nc.sync.dma_start(
                        out=out_d.ap()[mt * 128:(mt + 1) * 128,
                                       sh * 1024:(sh + 1) * 1024],
                        in_=o_sb)

        # ---------- emission schedule ----------
        # group order qh-major: (p0,q0), (p1,q0), (p0,q1), (p1,q1)
        GROUPS = [(0, 0), (1, 0), (0, 1), (1, 1)]

        # K0/Q0 t=0 unblock attention group 0 after only a quarter of the
        # Q/K projection work (and only the s<1024 xt halves).
        pair0, qh0 = GROUPS[0]
        g0_pts = []
        qk_flight_t(1, k_sb, 0, 2, 0)
        qk_flight_t(0, q_sb, 0, 0, 0)
        for kc in range(8):
            g0_pts.append(qk_exp(pair0, qh0, kc))
        qk_flight_t(1, k_sb, 0, 2, 1)
        for kc in range(8, N_KC):
            g0_pts.append(qk_exp(pair0, qh0, kc))
        qk_flight_t(0, q_sb, 0, 0, 1)
        qk_flight_t(1, k_sb, 1, 3, 0)
        qk_flight_t(1, k_sb, 1, 3, 1)
        qk_flight_t(0, q_sb, 1, 1, 0)
        qk_flight_t(0, q_sb, 1, 1, 1)
        for scg in range(4):
            v_flight(scg)

        # group 0 PV + normalize
        acc_a = accp.tile([DK + 1, QH], F32, tag="acc")
        acc_b = accp.tile([DK + 1, QH], F32, tag="acc")
        accs = [acc_a, acc_b]
        for kc in range(N_KC):
            pv(pair0, kc, g0_pts[kc], accs)
        evict_and_normalize(0, pair0, qh0, accs)

        # groups 1-3: fully interleaved steady state
        for gi in range(1, 4):
            pair, qh = GROUPS[gi]
            acc_a = accp.tile([DK + 1, QH], F32, tag="acc")
            acc_b = accp.tile([DK + 1, QH], F32, tag="acc")
            accs = [acc_a, acc_b]
            for kc in range(N_KC):
                pt = qk_exp(pair, qh, kc)
                pv(pair, kc, pt, accs)
            evict_and_normalize(gi, pair, qh, accs)

        wo_phase()

    nc.compile()
    return nc


def make_core_inputs(x, Wq, bq, Wk, bk, Wv, bv, Wo, bo, rel_bias):
    """Host-side shard prep. Returns list of 8 in_maps."""
    bf16 = ml_dtypes.bfloat16
    x = np.asarray(x, np.float32)
    in_maps = []
    WqT = np.ascontiguousarray(np.asarray(Wq, np.float32).T.astype(bf16))
    WkT = np.ascontiguousarray(np.asarray(Wk, np.float32).T.astype(bf16))
    WvT = np.ascontiguousarray(np.asarray(Wv, np.float32).T.astype(bf16))
    WoT = np.ascontiguousarray(np.asarray(Wo, np.float32).T.astype(bf16))
    rel = np.asarray(rel_bias, np.float32)
    xt = [np.ascontiguousarray(x[b].T.astype(bf16)) for b in range(B)]

    # band multiplier: [p, h_local, m] = exp(bias(q,k) - c_past), q-k = m-128-p
    p_i = np.arange(128)[:, None]
    m_i = np.arange(BAND)[None, :]
    delta = np.clip(m_i - 128 - p_i, -MAX_REL, MAX_REL) + MAX_REL  # [128, 384]

    for c in range(N_CORES):
        b = c // CORES_PER_BATCH
        g = c % CORES_PER_BATCH
        c0 = g * CL
        heads = np.arange(g * HEADS_PER_CORE, (g + 1) * HEADS_PER_CORE)

        bqk = np.empty((128, 4), np.float32)
        bqk[:, 0] = np.asarray(bq, np.float32)[c0:c0 + 128]
        bqk[:, 1] = np.asarray(bq, np.float32)[c0 + 128:c0 + 256]
        bqk[:, 2] = np.asarray(bk, np.float32)[c0:c0 + 128]
        bqk[:, 3] = np.asarray(bk, np.float32)[c0 + 128:c0 + 256]

        band = np.empty((128, HEADS_PER_CORE, BAND), np.float32)
        fmult = np.empty((128, HEADS_PER_CORE), np.float32)
        for i, hg in enumerate(heads):
            c_past = rel[hg, 2 * MAX_REL]
            band[:, i, :] = np.exp(rel[hg][delta] - c_past)
            fmult[:, i] = np.exp(rel[hg, 0] - c_past)  # future multiplier
        wqkv = np.stack([WqT[:, c0:c0 + CL], WkT[:, c0:c0 + CL],
                         WvT[:, c0:c0 + CL]], axis=1)
        in_maps.append({
            "xt": xt[b],
            "wqkv": np.ascontiguousarray(wqkv),
            "wot": np.ascontiguousarray(WoT[c0:c0 + CL, :]),
            "bqk": bqk,
            "band": band.astype(bf16),
            "fmult": fmult,
        })
    return in_maps


_NC_CACHE = {}


def get_program(**kw):
    key = tuple(sorted(kw.items()))
    if key not in _NC_CACHE:
        _NC_CACHE[key] = build_program(**kw)
    return _NC_CACHE[key]


def kernel(x, Wq, bq, Wk, bk, Wv, bv, Wo, bo, rel_bias):
    from concourse.bass_utils import run_bass_kernel_spmd

    nc = get_program()
    in_maps = make_core_inputs(x, Wq, bq, Wk, bk, Wv, bv, Wo, bo, rel_bias)
    res = run_bass_kernel_spmd(nc, in_maps, core_ids=list(range(N_CORES)))
    results = res.results

    Wo_np = np.asarray(Wo, np.float32)
    const = np.asarray(bv, np.float32) @ Wo_np.T + np.asarray(bo, np.float32)
    out = np.zeros((B, S, D), np.float32)
    for c in range(N_CORES):
        out[c // CORES_PER_BATCH] += results[c]["out_p"].astype(np.float32).T
    out += const[None, None, :]
    return out


# revision 32
# speedup vs baseline: 1.1552x; 1.1357x over previous
"""Trainium2 Bass kernel for MultiHeadAttention with relative position bias.

Reference computation (B=2, S=2048, D=1024, H=16, Dk=64, MAX_REL=128):
    Q,K,V = x@W{q,k,v}.T + b      (per-head reshape)
    scores = QK^T/sqrt(Dk) + rel_bias_matrix
    out = softmax(scores) @ V, heads merged, @ Wo.T + bo

Sharding (8 cores): core c handles batch b=c//4 and 4 heads hg=4*(c%4)..+4
(data + head parallel). Q/K/V projections column-split per head group,
Wo row-split; the partial outputs are summed on the host (the "all-reduce").

Pipeline design:
  The exp stream on the Scalar/ACT engine (16.8M elem/core at 1 elem/
  cycle/lane @1.2GHz, ~142us) and the PE matmul stream (~200us incl the
  per-MM weight-load tax) are the two co-bottlenecks; the schedule keeps
  both fed.  All tensor data is bf16 (final rel err ~6e-3 vs 2e-2
  tolerance): halves input DMA, enables FWL weight loads, shrinks SBUF.
  PSUM budget: stp pool (2x[128,1024] slots, 4 banks) for QK scores +
  Wo tiles; accp pool (2 slots, 4 banks) for projection flights + PV
  accumulators.

  Emission order: input DMAs (xt split into 16 chunk DMAs - one
  dma_start lands on ~one queue at ~24GB/s, so parallelism needs many
  in flight) -> K0/Q0 flights -> group0 QK+exp (pt tiles buffered in a
  17-deep pool) -> K1/Q1/V flights -> group0 PV + normalize -> groups
  1-3 fully interleaved -> transposed-Wo phase.  Group order qh-major.

  Softmax trick: P~ = exp(s/8) is the softmax numerator up to the
  per-head constant e^{-c_past} which cancels in the normalize; the
  "future" region (q-k <= -128) gets a constant multiplier on DVE and
  the 384-wide Toeplitz band a host-precomputed exp(bias - c_past)
  tile.  V carries a ones column so PV yields the denominator for free;
  normalize uses reciprocal_approx_fast (5x DVE reciprocal) on a
  DRAM-broadcast denominator.  Output is emitted transposed [D,S] in
  bf16 (Wo stationary => LDW amortized; halved out-DMA) and the host
  upcasts/transposes/sums.
"""

import math
import os
import sys

for _p in ("/opt/trn_rl_repo", "/root/.axon_site", "/root/.axon_site/_ro/trn_rl_repo",
           "/root/.axon_site/_ro/pypackages"):
    if os.path.isdir(_p) and _p not in sys.path:
        sys.path.append(_p)

import numpy as np
import ml_dtypes

import concourse.bass as bass
import concourse.mybir as mybir
import concourse.tile as tile
from concourse import bacc
from contextlib import ExitStack

# Problem constants (hardcoded per the contract).
B, S, D = 2, 2048, 1024
H, DK = 16, 64
MAX_REL = 128
N_CORES = 8
CORES_PER_BATCH = 4
HEADS_PER_CORE = H // CORES_PER_BATCH  # 4
CL = HEADS_PER_CORE * DK               # 256 local channels
N_PAIRS = HEADS_PER_CORE // 2          # 2 head pairs
QH = 1024                              # q processed in halves
N_QH = S // QH                         # 2
N_KC = S // 128                        # 16 k chunks
NDC = D // 128                         # 8 contraction chunks
BAND = 3 * 128                         # band width in q for one k chunk

F32 = mybir.dt.float32
BF16 = mybir.dt.bfloat16

SCALE = 1.0 / math.sqrt(DK)
EXP = mybir.ActivationFunctionType.Exp


def build_program():
    nc = bacc.Bacc("TRN2", target_bir_lowering=False, debug=False)

    xt_d = nc.declare_dram_parameter("xt", [D, S], BF16, isOutput=False)
    wqt_d = nc.declare_dram_parameter("wqt", [D, CL], BF16, isOutput=False)
    wkt_d = nc.declare_dram_parameter("wkt", [D, CL], BF16, isOutput=False)
    wvt_d = nc.declare_dram_parameter("wvt", [D, CL], BF16, isOutput=False)
    wot_d = nc.declare_dram_parameter("wot", [CL, D], BF16, isOutput=False)
    bqk_d = nc.declare_dram_parameter("bqk", [128, 4], F32, isOutput=False)
    band_d = nc.declare_dram_parameter("band", [128, HEADS_PER_CORE, BAND], BF16,
                                       isOutput=False)
    # future-region multiplier exp(c_fut - c_past), replicated over partitions
    fmult_d = nc.declare_dram_parameter("fmult", [128, HEADS_PER_CORE], F32,
                                        isOutput=False)
    # partial output, TRANSPOSED [D, S] in bf16 (host upcasts + transposes)
    out_d = nc.declare_dram_parameter("out_p", [D, S], BF16, isOutput=True)
    # denominator round-trip scratch: [group, hh, q]
    den_d = nc.dram_tensor("den_scratch", [2 * N_PAIRS * N_QH, 2, QH], F32)

    with tile.TileContext(nc) as tc, ExitStack() as ctx:
        # ---------- long-lived SBUF ----------
        persist = ctx.enter_context(tc.tile_pool(name="persist", bufs=1))
        q_sb = persist.tile([128, 2, S], BF16, tag="q_sb")
        k_sb = persist.tile([128, 2, S], BF16, tag="k_sb")
        v_sb = persist.tile([128, N_KC, HEADS_PER_CORE, DK + 1], BF16, tag="v_sb")
        ct_sb = persist.tile([128, 2, S], BF16, tag="ct_sb")
        wo_sb = persist.tile([128, 2, D], BF16, tag="wo_sb")
        band_sb = persist.tile([128, HEADS_PER_CORE, BAND], BF16, tag="band_sb")
        bqk_sb = persist.tile([128, 4], F32, tag="bqk_sb")
        fmult_sb = persist.tile([128, HEADS_PER_CORE], F32, tag="fmult_sb")
        warm_sb = persist.tile([128, 16], F32, tag="warm_sb")

        xw = ctx.enter_context(tc.tile_pool(name="xw", bufs=1))
        xt_sb = xw.tile([128, NDC, S], BF16, tag="xt_sb")
        wq_sb = xw.tile([128, NDC, CL], BF16, tag="wq_sb")
        wk_sb = xw.tile([128, NDC, CL], BF16, tag="wk_sb")
        wv_sb = xw.tile([128, NDC, CL], BF16, tag="wv_sb")

        nrm = ctx.enter_context(tc.tile_pool(name="nrm", bufs=2))
        ptp = ctx.enter_context(tc.tile_pool(name="ptp", bufs=17))
        outp = ctx.enter_context(tc.tile_pool(name="outp", bufs=3))

        # ---------- PSUM pools: 4 banks each ----------
        stp = ctx.enter_context(tc.tile_pool(name="stp", bufs=2, space="PSUM"))
        accp = ctx.enter_context(tc.tile_pool(name="accp", bufs=2, space="PSUM"))

        # ---------- input DMAs, interleaved for early start ----------
        xt_v = xt_d.ap().rearrange("(c p) s -> p c s", p=128)
        wq_v = wqt_d.ap().rearrange("(c p) m -> p c m", p=128)
        wk_v = wkt_d.ap().rearrange("(c p) m -> p c m", p=128)
        wv_v = wvt_d.ap().rearrange("(c p) m -> p c m", p=128)

        # ACT exp-table warmup (overlaps the input DMA wait)
        nc.vector.memset(warm_sb, 0.0)
        nc.scalar.activation(out=warm_sb, in_=warm_sb, func=EXP, scale=1.0)

        # weights first (small), then xt split into 16 half-chunk DMAs so all
        # 16 DMA queues stream in parallel (~24 GB/s per queue)
        nc.sync.dma_start(out=wk_sb, in_=wk_v)
        nc.sync.dma_start(out=wq_sb, in_=wq_v)
        for dc in range(NDC):
            for h in range(2):
                nc.sync.dma_start(out=xt_sb[:, dc, h * 1024:(h + 1) * 1024],
                                  in_=xt_v[:, dc, h * 1024:(h + 1) * 1024])
        nc.sync.dma_start(out=wv_sb, in_=wv_v)
        nc.sync.dma_start(out=band_sb, in_=band_d.ap())
        nc.sync.dma_start(out=bqk_sb, in_=bqk_d.ap())
        nc.sync.dma_start(out=fmult_sb, in_=fmult_d.ap())
        nc.sync.dma_start(out=wo_sb, in_=wot_d.ap().rearrange("(c p) m -> p c m", p=128))
        nc.vector.memset(v_sb[:, :, :, DK:DK + 1], 1.0)

        # ---------- projection flights (dc-outer: LDW amortized 4x) ----------
        def qk_flight(w_sb, dst_sb, j, bias_col):
            slot_a = accp.tile([128, 1024], F32, tag="acc")
            slot_b = accp.tile([128, 1024], F32, tag="acc")
            slots = [slot_a, slot_b]
            for dc in range(NDC):
                for t in range(2):
                    for half in range(2):
                        nc.tensor.matmul(
                            slots[t][:, half * 512:(half + 1) * 512],
                            lhsT=w_sb[:, dc, j * 128:(j + 1) * 128],
                            rhs=xt_sb[:, dc, t * 1024 + half * 512:
                                      t * 1024 + (half + 1) * 512],
                            start=(dc == 0), stop=(dc == NDC - 1),
                        )
            for t in range(2):
                nc.vector.tensor_scalar_add(
                    out=dst_sb[:, j, t * 1024:(t + 1) * 1024],
                    in0=slots[t],
                    scalar1=bqk_sb[:, bias_col:bias_col + 1],
                )

        def v_flight(scg):
            # each 256-wide accumulation group must own a full PSUM bank
            # (start=True clears has_written for the whole bank), so 4
            # s-chunks land at 512-col boundaries across two slots.
            slot_a = accp.tile([128, 1024], F32, tag="acc")
            slot_b = accp.tile([128, 1024], F32, tag="acc")
            both = (slot_a, slot_b)
            for dc in range(NDC):
                for i in range(4):
                    sc = scg * 4 + i
                    nc.tensor.matmul(
                        both[i // 2][:, (i % 2) * 512:(i % 2) * 512 + CL],
                        lhsT=xt_sb[:, dc, sc * 128:(sc + 1) * 128],
                        rhs=wv_sb[:, dc, :],
                        start=(dc == 0), stop=(dc == NDC - 1),
                    )
            for i in range(4):
                sc = scg * 4 + i
                # ACT copy: ScalarE is idle during the projection era and
                # reads PSUM faster than DVE does
                nc.scalar.copy(
                    out=v_sb[:, sc, :, 0:DK],
                    in_=both[i // 2][:, (i % 2) * 512:(i % 2) * 512 + CL]
                    .rearrange("p (h d) -> p h d", h=HEADS_PER_CORE),
                )

        # ---------- attention pieces ----------
        def qk_exp(pair, qh, kc):
            """QK matmuls + exp + band/future fixups; returns the pt tile."""
            w0 = qh * QH
            k0 = kc * 128
            pt = ptp.tile([128, 2, QH], BF16, tag="pt")
            for hh in range(2):
                h = 2 * pair + hh
                p0 = hh * 64
                st = stp.tile([128, QH], F32, tag="st")
                for half in range(2):
                    nc.tensor.matmul(
                        st[:, half * 512:(half + 1) * 512],
                        lhsT=k_sb[p0:p0 + 64, pair, k0:k0 + 128],
                        rhs=q_sb[p0:p0 + 64, pair,
                                 w0 + half * 512:w0 + (half + 1) * 512],
                        start=True, stop=True,
                        tile_position=(p0, 0),
                    )
                nc.scalar.activation(out=pt[:, hh, :], in_=st, func=EXP, scale=SCALE)
                # future region (q <= k0-129): multiply by exp(c_fut - c_past)
                fut_end = min(max(k0 - 128, w0), w0 + QH)
                n_fut = fut_end - w0
                if n_fut > 0:
                    nc.vector.tensor_scalar_mul(
                        out=pt[:, hh, 0:n_fut], in0=pt[:, hh, 0:n_fut],
                        scalar1=fmult_sb[:, h:h + 1],
                    )
                # band: q in [k0-128, k0+256) -> multiply exp(bias - c_past)
                b_lo = max(k0 - 128, w0)
                b_hi = min(k0 + 2 * 128, w0 + QH)
                if b_hi > b_lo:
                    m0 = b_lo - (k0 - 128)
                    nc.vector.tensor_mul(
                        out=pt[:, hh, b_lo - w0:b_hi - w0],
                        in0=pt[:, hh, b_lo - w0:b_hi - w0],
                        in1=band_sb[:, h, m0:m0 + (b_hi - b_lo)],
                    )
            return pt

        def pv(pair, kc, pt, accs):
            for hh in range(2):
                for sub in range(2):
                    nc.tensor.matmul(
                        accs[hh][:, sub * 512:(sub + 1) * 512],
                        lhsT=v_sb[:, kc, 2 * pair + hh, :],
                        rhs=pt[:, hh, sub * 512:(sub + 1) * 512],
                        start=(kc == 0), stop=(kc == N_KC - 1),
                    )

        def evict_and_normalize(gi, pair, qh, accs):
            w0 = qh * QH
            den_sb = nrm.tile([1, 2, QH], F32, tag="den")
            for hh in range(2):
                nc.vector.tensor_copy(
                    out=ct_sb[hh * 64:hh * 64 + 64, pair, w0:w0 + QH],
                    in_=accs[hh][0:DK, :])
                nc.vector.tensor_copy(
                    out=den_sb[:, hh, :],
                    in_=accs[hh][DK:DK + 1, :])
            den_v = den_d.ap()
            nc.sync.dma_start(out=den_v[gi], in_=den_sb)
            rbc = nrm.tile([128, QH], F32, tag="rbc")
            for hh in range(2):
                bsrc = bass.AP(
                    tensor=den_v.tensor,
                    offset=den_v.offset + (gi * 2 + hh) * QH,
                    ap=[[0, 64], [1, QH]],
                )
                nc.sync.dma_start(out=rbc[hh * 64:hh * 64 + 64, :], in_=bsrc)
            nc.vector.reciprocal_approx_fast(out=rbc, in_=rbc)
            nc.vector.tensor_mul(
                out=ct_sb[:, pair, w0:w0 + QH],
                in0=ct_sb[:, pair, w0:w0 + QH],
                in1=rbc,
            )

        def wo_phase():
            # transposed output out^T[m, s]: Wo slices are the stationary
            # operand (LDW amortized 4x: one load covers both s-halves), ct
            # streams.  Evictions on ACT (idle in the tail), bf16 partials
            # halve the out-DMA bytes.
            for mt in range(8):
                ps0 = stp.tile([128, 1024], F32, tag="st")
                ps1 = stp.tile([128, 1024], F32, tag="st")
                both = (ps0, ps1)
                for j in range(2):
                    for sh in range(2):
                        for ss in range(2):
                            nc.tensor.matmul(
                                both[sh][:, ss * 512:(ss + 1) * 512],
                                lhsT=wo_sb[:, j, mt * 128:(mt + 1) * 128],
                                rhs=ct_sb[:, j, sh * 1024 + ss * 512:
                                          sh * 1024 + (ss + 1) * 512],
                                start=(j == 0), stop=(j == 1),
                            )
                for sh in range(2):
                    o_sb = outp.tile([128, 1024], BF16, tag="o_sb")
                    nc.scalar.copy(out=o_sb, in_=both[sh])
                    nc.sync.dma_start(
                        out=out_d.ap()[mt * 128:(mt + 1) * 128,
                                       sh * 1024:(sh + 1) * 1024],
                        in_=o_sb)

        # ---------- emission schedule ----------
        # group order qh-major: (p0,q0), (p1,q0), (p0,q1), (p1,q1)
        GROUPS = [(0, 0), (1, 0), (0, 1), (1, 1)]

        qk_flight(wk_sb, k_sb, 0, 2)
        qk_flight(wq_sb, q_sb, 0, 0)

        # group 0: QK+exp only (PSUM accs still busy with projections);
        # pt tiles buffer in the deep ptp pool until PV catches up.
        g0_pts = []
        pair0, qh0 = GROUPS[0]
        for kc in range(N_KC):
            g0_pts.append(qk_exp(pair0, qh0, kc))

        qk_flight(wk_sb, k_sb, 1, 3)
        qk_flight(wq_sb, q_sb, 1, 1)
        for scg in range(4):
            v_flight(scg)

        # group 0 PV + normalize
        acc_a = accp.tile([DK + 1, QH], F32, tag="acc")
        acc_b = accp.tile([DK + 1, QH], F32, tag="acc")
        accs = [acc_a, acc_b]
        for kc in range(N_KC):
            pv(pair0, kc, g0_pts[kc], accs)
        evict_and_normalize(0, pair0, qh0, accs)

        # groups 1-3: fully interleaved steady state
        for gi in range(1, 4):
            pair, qh = GROUPS[gi]
            acc_a = accp.tile([DK + 1, QH], F32, tag="acc")
            acc_b = accp.tile([DK + 1, QH], F32, tag="acc")
            accs = [acc_a, acc_b]
            for kc in range(N_KC):
                pt = qk_exp(pair, qh, kc)
                pv(pair, kc, pt, accs)
            evict_and_normalize(gi, pair, qh, accs)

        wo_phase()

    nc.compile()
    return nc


def make_core_inputs(x, Wq, bq, Wk, bk, Wv, bv, Wo, bo, rel_bias):
    """Host-side shard prep. Returns list of 8 in_maps."""
    bf16 = ml_dtypes.bfloat16
    x = np.asarray(x, np.float32)
    in_maps = []
    WqT = np.ascontiguousarray(np.asarray(Wq, np.float32).T.astype(bf16))
    WkT = np.ascontiguousarray(np.asarray(Wk, np.float32).T.astype(bf16))
    WvT = np.ascontiguousarray(np.asarray(Wv, np.float32).T.astype(bf16))
    WoT = np.ascontiguousarray(np.asarray(Wo, np.float32).T.astype(bf16))
    rel = np.asarray(rel_bias, np.float32)
    xt = [np.ascontiguousarray(x[b].T.astype(bf16)) for b in range(B)]

    # band multiplier: [p, h_local, m] = exp(bias(q,k) - c_past), q-k = m-128-p
    p_i = np.arange(128)[:, None]
    m_i = np.arange(BAND)[None, :]
    delta = np.clip(m_i - 128 - p_i, -MAX_REL, MAX_REL) + MAX_REL  # [128, 384]

    for c in range(N_CORES):
        b = c // CORES_PER_BATCH
        g = c % CORES_PER_BATCH
        c0 = g * CL
        heads = np.arange(g * HEADS_PER_CORE, (g + 1) * HEADS_PER_CORE)

        bqk = np.empty((128, 4), np.float32)
        bqk[:, 0] = np.asarray(bq, np.float32)[c0:c0 + 128]
        bqk[:, 1] = np.asarray(bq, np.float32)[c0 + 128:c0 + 256]
        bqk[:, 2] = np.asarray(bk, np.float32)[c0:c0 + 128]
        bqk[:, 3] = np.asarray(bk, np.float32)[c0 + 128:c0 + 256]

        band = np.empty((128, HEADS_PER_CORE, BAND), np.float32)
        fmult = np.empty((128, HEADS_PER_CORE), np.float32)
        for i, hg in enumerate(heads):
            c_past = rel[hg, 2 * MAX_REL]
            band[:, i, :] = np.exp(rel[hg][delta] - c_past)
            fmult[:, i] = np.exp(rel[hg, 0] - c_past)  # future multiplier
        in_maps.append({
            "xt": xt[b],
            "wqt": np.ascontiguousarray(WqT[:, c0:c0 + CL]),
            "wkt": np.ascontiguousarray(WkT[:, c0:c0 + CL]),
            "wvt": np.ascontiguousarray(WvT[:, c0:c0 + CL]),
            "wot": np.ascontiguousarray(WoT[c0:c0 + CL, :]),
            "bqk": bqk,
            "band": band.astype(bf16),
            "fmult": fmult,
        })
    return in_maps


_NC_CACHE = {}


def get_program(**kw):
    key = tuple(sorted(kw.items()))
    if key not in _NC_CACHE:
        _NC_CACHE[key] = build_program(**kw)
    return _NC_CACHE[key]


def kernel(x, Wq, bq, Wk, bk, Wv, bv, Wo, bo, rel_bias):
    from concourse.bass_utils import run_bass_kernel_spmd

    nc = get_program()
    in_maps = make_core_inputs(x, Wq, bq, Wk, bk, Wv, bv, Wo, bo, rel_bias)
    res = run_bass_kernel_spmd(nc, in_maps, core_ids=list(range(N_CORES)))
    results = res.results

    Wo_np = np.asarray(Wo, np.float32)
    const = np.asarray(bv, np.float32) @ Wo_np.T + np.asarray(bo, np.float32)
    out = np.zeros((B, S, D), np.float32)
    for c in range(N_CORES):
        out[c // CORES_PER_BATCH] += results[c]["out_p"].astype(np.float32).T
    out += const[None, None, :]
    return out
